# revision 111
# baseline (speedup 1.0000x reference)
"""PlatonicConv (graph-mode attention) Trainium2 Bass kernel.

Math (per graph of 64 fully-connected nodes, 24 group-heads of dim 16):
  q/k/v = x @ W; RoPE(q, k) from pos; S = q.k^T/4; softmax over dst;
  out = A @ v; y = out @ Wo.  32 graphs -> data-parallel over 8 cores.

Layout choices (per core: 4 graphs, 256 nodes):
  * Attention side lives transposed ([feature, node]); x is transposed on host.
  * RoPE cos/sin caches are host-precomputed patterns [128, 256]; the rotation
    partner-swap + sign is folded into a second spread matrix (esp2b), so the
    pair-swapped projections (Wqp/Wkp) are not needed: only one projection per
    q/k slab, then  spread = esp2a^T (qt*cos) + esp2b^T (qt*sin)  accumulated
    in PSUM (this also absorbs the rope add).
  * Heads are "spread" to 32-aligned partition slots so score matmuls pack
    4-way into the PE array via tile_position row groups.
  * Softmax is max-free (scores are O(1) by construction); the denominator
    comes for free as a 17th row of each AV matmul via an interleaved
    ones-column in the V weights.
  * The spread->compact "unspread" after AV is a single selection matmul per
    head-quad (parity-matched selE/selO place rows so every engine move is
    partition-shift-free); softmax denominators are Ln'd out of PSUM by the
    scalar engine, broadcast to emb rows by a tiny e68 matmul, and applied
    as exp(-ln den) -- no flat-cost DVE reciprocal anywhere.
  * Projections/V/output weights, x^T and the attention internals run in
    bf16 (tol is 2e-2); psum accumulation stays f32.
  * Most tensors ride single-descriptor DMAs split across the two HWDGE
    queues; x^T shares one tensor with Wq so the first projection waits on
    exactly one transfer.

HW lessons baked in: f32r operands forbid tile_position dst offsets >= 64;
one OPEN psum accumulation group per 2KB bank at a time (a [128,512] f32
tile with two open groups deadlocks hardware, sim passes); DVE reciprocal
costs ~1.7us flat regardless of size; engines cannot shift partitions
(only DMA/PE can); act-table loads (~1.3us) fire on every scalar-engine
function switch.
"""

import numpy as np

G = 12
H = 2
D = 16
GH = 24          # G * H group-heads
C = 384          # in/emb/out channels
NG = 32          # graphs
NPG = 64         # nodes per graph
N = NG * NPG
NCORES = 8
GPC = NG // NCORES   # graphs per core = 4
NPC = GPC * NPG      # nodes per core = 256
VW = 17              # V block width (16 + ones col)
CAUG = GH * VW       # 408
CSTW = 1116          # packed consts width

_CACHE = {}


def _host_prep(Wq, Wk, Wv, Wo, rope_freqs):
    import ml_dtypes
    f32 = np.float32

    # esp2a: compact row (within 64-block) -> 32-aligned spread slot; stacked
    # twice so odd 64-row slabs can use base partition 64
    esp2a = np.zeros((128, 128), f32)
    for k in range(64):
        m = 32 * (k // 16) + (k % 16)
        esp2a[k, m] = 1.0
        esp2a[64 + k, m] = 1.0
    # esp2b = P^T esp2a: P = rope pair-swap (d even<->odd) with sign(-1 on even)
    p64 = np.arange(64) ^ 1
    s64 = np.where(np.arange(64) % 2 == 0, -1.0, 1.0).astype(f32)
    esp2b = np.zeros((128, 128), f32)
    for c in range(64):
        r = p64[c]
        esp2b[c, :] = s64[r] * esp2a[r, :]
        esp2b[64 + c, :] = s64[r] * esp2a[64 + r, :]

    # unspread selection, parity-matched so every downstream engine move is
    # partition-shift-free:
    #   even quads: og -> out rows 0-63, den -> rows 64-67
    #   odd quads:  og -> out rows 64-127, den -> rows 0-3
    selE = np.zeros((128, 68), f32)
    selO = np.zeros((128, 128), f32)
    for a in range(4):
        selE[32 * a + 16, 64 + a] = 1.0
        selO[32 * a + 16, a] = 1.0
        for d in range(16):
            selE[32 * a + d, 16 * a + d] = 1.0
            selO[32 * a + d, 64 + 16 * a + d] = 1.0

    # e68: rden2 rows (64-67 = even-quad dens -> out rows 0-63; 0-3 = odd-quad
    # dens -> out rows 64-127), broadcast to 16 consecutive emb rows each
    e68 = np.zeros((68, 128), f32)
    for i in range(64):
        e68[64 + i // 16, i] = 1.0
        e68[i // 16, 64 + i] = 1.0

    # packed consts [128, 1052]
    cst = np.zeros((128, CSTW), ml_dtypes.bfloat16)
    cst[:, 0:128] = esp2a
    cst[:, 128:256] = esp2b
    cst[:, 256:324] = selE
    cst[:, 324:452] = selO
    cst[0:68, 452:580] = e68
    cst[0, 580:708] = 1.0                        # onesrow
    cst[0, 708 + VW * np.arange(GH) + 16] = 1.0  # vseed (cols 708:1116)

    # V interleaved with a ones column per head: block j = [Wv head j | 0]
    Wvil = np.zeros((C, CAUG), f32)
    for j in range(GH):
        Wvil[:, VW * j:VW * j + 16] = Wv[:, 16 * j:16 * j + 16]

    bf16 = ml_dtypes.bfloat16

    def pack(w):
        # [384, cols] -> [128, 3*cols]: col block s = w[128 s : 128 s + 128]
        cols = w.shape[1]
        return np.ascontiguousarray(
            w.reshape(3, 128, cols).transpose(1, 0, 2).reshape(128, 3 * cols)
            .astype(bf16))

    wko = np.concatenate([pack(Wk).reshape(128, 3, C),
                          pack(Wo).reshape(128, 3, C)], axis=2)
    return dict(
        wko=np.ascontiguousarray(wko.reshape(128, 3 * 2 * C)),
        wvil=pack(Wvil), cst=cst,
        e68=np.ascontiguousarray(e68.astype(ml_dtypes.bfloat16)),
        _wq=pack(Wq).reshape(128, 3, C),
    )


def _rope_cache(pos, rope_freqs):
    # cos/sin patterns [128, 256]: row r (mod 64) = 16 m + d -> head h = m%2,
    # freq index d//2; two stacked 64-row copies
    f32 = np.float32
    theta = np.einsum('ns,shf->nhf', pos.astype(f32), rope_freqs.astype(f32))
    r = np.arange(64)
    h = (r // 16) % 2
    f = (r % 16) // 2
    cpat = np.cos(theta[:, h, f]).T.astype(f32)   # [64, 256]
    spat = np.sin(theta[:, h, f]).T.astype(f32)
    import ml_dtypes
    cs = np.empty((128, 2 * NPC), ml_dtypes.bfloat16)
    cs[0:64, 0:NPC] = cpat
    cs[64:128, 0:NPC] = cpat
    cs[0:64, NPC:] = spat
    cs[64:128, NPC:] = spat
    return cs


def _build_nc():
    import concourse.bacc as bacc
    import concourse.tile as tile
    import concourse.mybir as mybir
    from contextlib import ExitStack

    f32 = mybir.dt.float32
    fmm = mybir.dt.float32r
    fb = mybir.dt.bfloat16
    AF = mybir.ActivationFunctionType

    nc = bacc.Bacc("TRN2", target_bir_lowering=False)

    # xtw: per-slab [xT_s | Wq_s] so one DMA feeds the first projections;
    # wko: per-slab [Wk_s | Wo_s]
    xtw_d = nc.dram_tensor("xtw", [128, 3 * (NPC + C)], fb, kind="ExternalInput")
    cs_d = nc.dram_tensor("cs", [128, 2 * NPC], fb, kind="ExternalInput")
    wko_d = nc.dram_tensor("wko", [128, 3 * 2 * C], fb, kind="ExternalInput")
    wvil_d = nc.dram_tensor("wvil", [128, 3 * CAUG], fb, kind="ExternalInput")
    cst_d = nc.dram_tensor("cst", [128, CSTW], fb, kind="ExternalInput")
    e68_d = nc.dram_tensor("e68", [68, 128], fb, kind="ExternalInput")
    y_d = nc.dram_tensor("y", [NPC, C], f32, kind="ExternalOutput")

    ctx = ExitStack()
    with tile.TileContext(nc) as tc, ctx:
        consts = ctx.enter_context(tc.tile_pool(name="consts", bufs=1))
        sb = ctx.enter_context(tc.tile_pool(name="sbuf", bufs=1))
        # general psum: shared tag -> recycled 1-bank slots
        ps_gp = ctx.enter_context(tc.tile_pool(name="ps_gp", bufs=2, space="PSUM"))
        ps_att = ctx.enter_context(tc.tile_pool(name="ps_att", bufs=1, space="PSUM"))
        ps_av = ctx.enter_context(tc.tile_pool(name="ps_av", bufs=1, space="PSUM"))

        def gpt(shape):
            return ps_gp.tile(shape, f32, tag="pp", name="pp")

        # ---- input DMAs: few large single-descriptor transfers, ordered by
        # first use, split across the two queues ----
        xtw = consts.tile([128, 3, NPC + C], fb, tag="xtw")
        nc.sync.dma_start(out=xtw,
                          in_=xtw_d.rearrange("p (s e) -> p s e", s=3))
        cs = consts.tile([128, 2, NPC], fb, tag="cs")
        nc.sync.dma_start(out=cs, in_=cs_d.rearrange("p (s e) -> p s e", s=2))
        wko = consts.tile([128, 3, 2 * C], fb, tag="wko")
        nc.scalar.dma_start(out=wko,
                            in_=wko_d.rearrange("p (s e) -> p s e", s=3))
        cst = consts.tile([128, CSTW], fb, tag="cst")
        nc.scalar.dma_start(out=cst, in_=cst_d[:])
        wvil = consts.tile([128, 3, CAUG], fb, tag="wvil")
        nc.sync.dma_start(out=wvil,
                          in_=wvil_d.rearrange("p (s e) -> p s e", s=3))
        e68 = consts.tile([68, 128], fb, tag="e68")
        nc.scalar.dma_start(out=e68, in_=e68_d[:])

        cosf = cs[:, 0, :]
        sinf = cs[:, 1, :]
        esp2a = cst[:, 0:128]
        esp2b = cst[:, 128:256]
        selE = cst[:, 256:324]
        selO = cst[:, 324:452]
        onesrow = cst[0:1, 580:708]
        vseed = cst[0:1, 708:708 + CAUG]

        # ---- projections (transposed) + RoPE + spread, per 128-row m-slab.
        # spread = esp2a^T (qt*cos) + esp2b^T (qt*sin), accumulated in psum.
        # q/k interleaved per slab so the PE has projection work to do while
        # the DVE muls feed the spread matmuls; both 64-row halves land in
        # one single-bank [128, 512] psum tile -> one scalar copy each, and
        # the PE never waits on a copy for a psum slot.
        # Pair-0 scores for slab m-1 ride along after slab m.
        def proj(w, woff, m):
            ps = gpt([128, NPC])
            for k in range(3):
                nc.tensor.matmul(
                    out=ps,
                    lhsT=w[:, k, woff + 128 * m:woff + 128 * m + 128],
                    rhs=xtw[:, k, 0:NPC],
                    start=(k == 0), stop=(k == 2))
            return ps

        # qsp/ksp: per slab m one [128, 512] tile; tilei 2m+half -> col half
        def scol(gh):
            return 512 * (gh % 4) + 64 * (gh // 4)

        def sview(lst, tilei):
            return lst[tilei // 2][:, 256 * (tilei % 2):256 * (tilei % 2) + 256]

        def emit_scores(stp, pair, tiles):
            # Concurrent row-tiled matmuls MUST land in distinct PSUM banks:
            # head gh -> bank TILE stp[gh%4], col 64*(gh//4), rows 64*g01.
            # Per-bank tiles (vs one 4-bank tile) let pair-1 scores start as
            # soon as that bank's pair-0 exp has read it.
            for tilei in tiles:
                for slot in range(4):
                    gh = 4 * tilei + slot
                    lo = 32 * slot
                    col = 64 * (gh // 4)
                    for g01 in range(2):
                        g = 2 * pair + g01
                        nc.tensor.matmul(
                            out=stp[slot][64 * g01:64 * g01 + 64,
                                          col:col + 64],
                            lhsT=sview(ksp, tilei)[lo:lo + 16, 64 * g:64 * g + 64],
                            rhs=sview(qsp, tilei)[lo:lo + 16, 64 * g:64 * g + 64],
                            start=True, stop=True,
                            tile_position=(lo, 64 * g01))

        qsp, ksp = [], []
        vau = []
        stp0 = None
        for m in range(3):
            ab = {}
            for tag, woff in (("q", NPC), ("k", 0)):
                ps = proj(xtw if tag == "q" else wko, woff, m)
                a = sb.tile([128, NPC], fb, tag=f"ra{tag}{m}")
                b = sb.tile([128, NPC], fb, tag=f"rb{tag}{m}")
                nc.vector.tensor_mul(out=a, in0=ps, in1=cosf)
                nc.vector.tensor_mul(out=b, in0=ps, in1=sinf)
                ab[tag] = (a, b)
            for tag, lst in (("q", qsp), ("k", ksp)):
                a, b = ab[tag]
                # bf16 (not f32r): score matmuls use tile_position dst
                # offsets that are invalid for f32r operands
                t = sb.tile([128, 2 * NPC], fb, tag=f"sps{tag}{m}")
                for half in range(2):
                    hs = slice(64 * half, 64 * half + 64)
                    csl = slice(NPC * half, NPC * half + NPC)
                    # q spreads borrow the (idle until AV) ps_av banks so no
                    # spread matmul ever waits on a psum->sbuf copy for a slot
                    if tag == "q":
                        sp = ps_av.tile([128, NPC], f32, tag=f"av{half}",
                                        name="spq")
                    else:
                        sp = gpt([128, NPC])
                    nc.tensor.matmul(out=sp, lhsT=esp2a[hs, :],
                                     rhs=a[hs, :], start=True, stop=False)
                    nc.tensor.matmul(out=sp, lhsT=esp2b[hs, :],
                                     rhs=b[hs, :], start=False, stop=True)
                    if m == 2:
                        # last slab's copies on the DVE so the scalar engine
                        # sits parked at the score exps when scores finish
                        nc.vector.tensor_copy(out=t[:, csl], in_=sp)
                    else:
                        nc.scalar.activation(out=t[:, csl], in_=sp,
                                             func=AF.Copy)
                lst.append(t)
            if m == 0:
                # V_aug [256, 408] untransposed (+ ones cols via K=1 matmul);
                # fills PE slack while slab-0's spread copies drain
                for i in range(2):
                    ps = gpt([128, CAUG])
                    for k in range(3):
                        nc.tensor.matmul(
                            out=ps,
                            lhsT=xtw[:, k, 128 * i:128 * i + 128],
                            rhs=wvil[:, k, :],
                            start=(k == 0), stop=False)
                    nc.tensor.matmul(
                        out=ps, lhsT=onesrow, rhs=vseed,
                        start=False, stop=True)
                    t = sb.tile([128, CAUG], fb, tag=f"vau{i}")
                    nc.vector.tensor_copy(out=t, in_=ps)
                    vau.append(t)
            if m > 0:
                # pair-0 scores for slab m-1 ride along behind slab m, so
                # their qsp/ksp copies are long done when the PE gets here
                if stp0 is None:
                    stp0 = [ps_att.tile([128, 512], f32, tag=f"sb{b}", name="stp")
                            for b in range(4)]
                emit_scores(stp0, 0, (2 * (m - 1), 2 * (m - 1) + 1))

        # ---- remaining scores + exp per graph-pair ----
        emit_scores(stp0, 0, (4, 5))
        expst = []
        et0 = sb.tile([128, 4 * 512], fb, tag="expst0")
        for b in range(4):
            nc.scalar.activation(
                out=et0[:, 512 * b:512 * b + 384],
                in_=stp0[b][:, 0:384],
                func=AF.Exp, scale=0.25)
        expst.append(et0)
        stp1 = [ps_att.tile([128, 512], f32, tag=f"sb{b}", name="stp")
                for b in range(4)]
        emit_scores(stp1, 1, range(6))
        et1 = sb.tile([128, 4 * 512], fb, tag="expst1")
        for b in range(4):
            nc.scalar.activation(
                out=et1[:, 512 * b:512 * b + 384],
                in_=stp1[b][:, 0:384],
                func=AF.Exp, scale=0.25)
        expst.append(et1)

        # ---- AV (+den row): per (quad, parity) [128, 128] psum tiles so the
        # two concurrent row groups (graph parities) use distinct banks;
        # head slot 32*(gh%4) rows, col 64*(g//2).
        # Then unspread via one sel matmul: rows 0-3 = den, 4-67 = compact out.
        avsb = sb.tile([128, 6 * 256], fb, tag="avsb")
        og = [sb.tile([128, NPC], f32, tag=f"og{m}", name="og") for m in range(3)]
        # lden: ln(den); quad pair p -> cols 256p; even quad dens at rows
        # 64-67, odd at rows 0-3 (parity-matched to the Ln source partitions).
        # Rows 4-63 are never written but read by the e68 matmul: zero once.
        # (A DVE reciprocal costs ~1.7us flat, so normalization goes through
        # scalar Ln -> e68 broadcast matmul -> scalar Exp(scale=-1) instead.)
        lden = sb.tile([68, 3 * NPC], f32, tag="lden")
        # bf16 hi + bf16 residual of ln(den), produced per pair by the
        # otherwise-idle gpsimd engine -> the e68 broadcast matmuls run
        # single-pass bf16 (vs ~1.1us 4-pass f32 each on the tail)
        ldenH = sb.tile([68, 3 * NPC], fb, tag="ldenH")
        ldenL = sb.tile([68, 3 * NPC], fb, tag="ldenL")
        nc.vector.memset(lden, 0.0)
        # persistent per-parity AV psum tiles with 4 column regions (qd % 4)
        # -> 4-deep quad pipelining within 2 banks, no recycle stalls
        avt = [ps_av.tile([128, 512], f32, tag=f"av{g01}", name="av")
               for g01 in range(2)]
        nc.vector.memset(avt[0], 0.0)
        nc.vector.memset(avt[1], 0.0)
        ups_of = {}

        def finish(qd):
            # unspread + Ln + (pair boundary) og copies for quad qd; called
            # one quad late so the PE never idles waiting on the avsb casts
            odd = qd % 2
            cq = slice(256 * qd, 256 * qd + 256)
            ups = gpt([128, NPC])
            nc.tensor.matmul(out=ups[0:68, :] if not odd else ups,
                             lhsT=selO if odd else selE, rhs=avsb[:, cq],
                             start=True, stop=True)
            ups_of[qd] = ups
            dhs = slice(64 - 64 * odd, 68 - 64 * odd)
            rp = slice(NPC * (qd // 2), NPC * (qd // 2) + NPC)
            nc.scalar.activation(out=lden[dhs, rp], in_=ups[dhs, :],
                                 func=AF.Ln)
            if odd:
                # node columns stay in (g01, pair, i) order (host gather
                # undoes it)
                nc.vector.tensor_copy(out=og[qd // 2][0:64, :],
                                      in_=ups_of[qd - 1][0:64, :])
                nc.vector.tensor_copy(out=og[qd // 2][64:128, :],
                                      in_=ups[64:128, :])
                # split this pair's ln(den) into bf16 hi + bf16 residual for
                # the single-pass rt matmuls: gpsimd (idle) for pairs 0/1,
                # but the DVE for pair 2 -- gpsimd's ~1.7us op chain would
                # sit on the tail, while the DVE is idle there
                if qd < 4:
                    nc.gpsimd.tensor_copy(out=ldenH[:, rp], in_=lden[:, rp])
                    nc.gpsimd.tensor_sub(out=ldenL[:, rp], in0=lden[:, rp],
                                         in1=ldenH[:, rp])
                else:
                    nc.vector.tensor_copy(out=ldenH[:, rp], in_=lden[:, rp])
                    nc.vector.scalar_tensor_tensor(
                        out=ldenL[:, rp], in0=ldenH[:, rp], scalar=-1.0,
                        in1=lden[:, rp],
                        op0=mybir.AluOpType.mult, op1=mybir.AluOpType.add)

        def av_half(qd, pair):
            reg = 128 * (qd % 4)
            for g01 in range(2):
                g = 2 * pair + g01
                lo = 64 * g01
                for a in range(4):
                    gh = 4 * qd + a
                    nc.tensor.matmul(
                        out=avt[g01][32 * a:32 * a + VW,
                                     reg + 64 * pair:reg + 64 * pair + 64],
                        lhsT=vau[pair][lo:lo + 64, VW * gh:VW * gh + VW],
                        rhs=expst[pair][lo:lo + 64, scol(gh):scol(gh) + 64],
                        start=True, stop=True,
                        tile_position=(lo, 32 * a))

        # pair-0 halves of quads 0-3 first: the PE grinds these while the
        # scalar engine is still serially producing the pair-1 exps
        for qd in range(4):
            av_half(qd, 0)
        for qd in range(6):
            if qd >= 4:
                av_half(qd, 0)
            av_half(qd, 1)
            reg = 128 * (qd % 4)
            for g01 in range(2):
                nc.vector.tensor_copy(
                    out=avsb[:, 256 * qd + 128 * g01:256 * qd + 128 * g01 + 128],
                    in_=avt[g01][:, reg:reg + 128])
            if qd > 0:
                finish(qd - 1)
        finish(5)

        # ---- normalize + y = O_norm @ Wo ----
        onrm = []
        for m in range(3):
            rt = gpt([128, NPC])
            rp = slice(NPC * m, NPC * m + NPC)
            nc.tensor.matmul(out=rt, lhsT=e68, rhs=ldenH[:, rp],
                             start=True, stop=False)
            nc.tensor.matmul(out=rt, lhsT=e68, rhs=ldenL[:, rp],
                             start=False, stop=True)
            ert = sb.tile([128, NPC], f32, tag=f"ert{m}")
            nc.scalar.activation(out=ert, in_=rt, func=AF.Exp, scale=-1.0)
            t = sb.tile([128, NPC], fb, tag=f"onrm{m}")
            nc.vector.tensor_mul(out=t, in0=og[m], in1=ert)
            onrm.append(t)

        for i in range(2):
            yps = gpt([128, C])
            for m in range(3):
                nc.tensor.matmul(
                    out=yps,
                    lhsT=onrm[m][:, 128 * i:128 * i + 128],
                    rhs=wko[:, m, C:2 * C],
                    start=(m == 0), stop=(m == 2))
            ysb = sb.tile([128, C], f32, tag=f"ysb{i}", name="ysb")
            if i == 0:
                nc.vector.tensor_copy(out=ysb, in_=yps)
            else:
                # scalar's Copy-table load hides behind the i=0 y matmuls
                nc.scalar.activation(out=ysb, in_=yps, func=AF.Copy)
            # contiguous DRAM write; the host gather undoes the node order
            # (y-tile i covers g01 == i, rows (pair, i64))
            nc.sync.dma_start(
                out=y_d.rearrange("(i r) e -> i r e", i=2)[i],
                in_=ysb)

    nc.compile()
    return nc


def _get_nc():
    if "nc" not in _CACHE:
        _CACHE["nc"] = _build_nc()
    return _CACHE["nc"]


def make_in_maps(inputs):
    x = np.asarray(inputs["x"], np.float32)
    pos = np.asarray(inputs["pos"], np.float32)
    freqs = np.asarray(inputs["rope_freqs"], np.float32)
    prep = _host_prep(np.asarray(inputs["Wq"], np.float32),
                      np.asarray(inputs["Wk"], np.float32),
                      np.asarray(inputs["Wv"], np.float32),
                      np.asarray(inputs["Wo"], np.float32),
                      freqs)
    in_maps = []
    import ml_dtypes
    wqp = prep.pop("_wq")
    for c in range(NCORES):
        sl = slice(c * NPC, (c + 1) * NPC)
        m = dict(prep)
        xs = x[sl].T                                # [384, 256]
        xTp = (xs.reshape(3, 128, NPC).transpose(1, 0, 2)
               .astype(ml_dtypes.bfloat16))         # [128, 3, 256]
        m["xtw"] = np.ascontiguousarray(
            np.concatenate([xTp, wqp], axis=2).reshape(128, 3 * (NPC + C)))
        m["cs"] = _rope_cache(pos[sl], freqs)
        in_maps.append(m)
    return in_maps


def gather(res):
    """Assemble the full [N, C] output; undoes the per-core (g01, pair, i64)
    node-column order the kernel keeps for contiguous DRAM writes."""
    outs = []
    for c in range(NCORES):
        yr = np.asarray(res.results[c]["y"], np.float32)   # [256, 384] raw
        outs.append(yr.reshape(2, 2, 64, C).transpose(1, 0, 2, 3).reshape(NPC, C))
    return np.concatenate(outs, axis=0)


def kernel(**inputs):
    from concourse.bass_utils import run_bass_kernel_spmd

    in_maps = make_in_maps(inputs)

    nc = _get_nc()
    res = run_bass_kernel_spmd(nc, in_maps, core_ids=list(range(NCORES)))
    return gather(res)


# revision 112
# speedup vs baseline: 1.0087x; 1.0087x over previous
"""PlatonicConv (graph-mode attention) Trainium2 Bass kernel.

Math (per graph of 64 fully-connected nodes, 24 group-heads of dim 16):
  q/k/v = x @ W; RoPE(q, k) from pos; S = q.k^T/4; softmax over dst;
  out = A @ v; y = out @ Wo.  32 graphs -> data-parallel over 8 cores.

Layout choices (per core: 4 graphs, 256 nodes):
  * Attention side lives transposed ([feature, node]); x is transposed on host.
  * RoPE cos/sin caches are host-precomputed patterns [128, 256]; the rotation
    partner-swap + sign is folded into a second spread matrix (esp2b), so the
    pair-swapped projections (Wqp/Wkp) are not needed: only one projection per
    q/k slab, then  spread = esp2a^T (qt*cos) + esp2b^T (qt*sin)  accumulated
    in PSUM (this also absorbs the rope add).
  * Heads are "spread" to 32-aligned partition slots so score matmuls pack
    4-way into the PE array via tile_position row groups.
  * Softmax is max-free (scores are O(1) by construction); the denominator
    comes for free as a 17th row of each AV matmul via an interleaved
    ones-column in the V weights.
  * The spread->compact "unspread" after AV is a single selection matmul per
    head-quad (parity-matched selE/selO place rows so every engine move is
    partition-shift-free); softmax denominators are Ln'd out of PSUM by the
    scalar engine, broadcast to emb rows by a tiny e68 matmul, and applied
    as exp(-ln den) -- no flat-cost DVE reciprocal anywhere.
  * Projections/V/output weights, x^T and the attention internals run in
    bf16 (tol is 2e-2); psum accumulation stays f32.
  * Most tensors ride single-descriptor DMAs split across the two HWDGE
    queues; x^T shares one tensor with Wq so the first projection waits on
    exactly one transfer.

HW lessons baked in: f32r operands forbid tile_position dst offsets >= 64;
one OPEN psum accumulation group per 2KB bank at a time (a [128,512] f32
tile with two open groups deadlocks hardware, sim passes); DVE reciprocal
costs ~1.7us flat regardless of size; engines cannot shift partitions
(only DMA/PE can); act-table loads (~1.3us) fire on every scalar-engine
function switch.
"""

import numpy as np

G = 12
H = 2
D = 16
GH = 24          # G * H group-heads
C = 384          # in/emb/out channels
NG = 32          # graphs
NPG = 64         # nodes per graph
N = NG * NPG
NCORES = 8
GPC = NG // NCORES   # graphs per core = 4
NPC = GPC * NPG      # nodes per core = 256
VW = 17              # V block width (16 + ones col)
CAUG = GH * VW       # 408
CSTW = 1116          # packed consts width

_CACHE = {}


def _host_prep(Wq, Wk, Wv, Wo, rope_freqs):
    import ml_dtypes
    f32 = np.float32

    # esp2a: compact row (within 64-block) -> 32-aligned spread slot; stacked
    # twice so odd 64-row slabs can use base partition 64
    esp2a = np.zeros((128, 128), f32)
    for k in range(64):
        m = 32 * (k // 16) + (k % 16)
        esp2a[k, m] = 1.0
        esp2a[64 + k, m] = 1.0
    # esp2b = P^T esp2a: P = rope pair-swap (d even<->odd) with sign(-1 on even)
    p64 = np.arange(64) ^ 1
    s64 = np.where(np.arange(64) % 2 == 0, -1.0, 1.0).astype(f32)
    esp2b = np.zeros((128, 128), f32)
    for c in range(64):
        r = p64[c]
        esp2b[c, :] = s64[r] * esp2a[r, :]
        esp2b[64 + c, :] = s64[r] * esp2a[64 + r, :]

    # unspread selection, parity-matched so every downstream engine move is
    # partition-shift-free:
    #   even quads: og -> out rows 0-63, den -> rows 64-67
    #   odd quads:  og -> out rows 64-127, den -> rows 0-3
    selE = np.zeros((128, 68), f32)
    selO = np.zeros((128, 128), f32)
    for a in range(4):
        selE[32 * a + 16, 64 + a] = 1.0
        selO[32 * a + 16, a] = 1.0
        for d in range(16):
            selE[32 * a + d, 16 * a + d] = 1.0
            selO[32 * a + d, 64 + 16 * a + d] = 1.0

    # e68: rden2 rows (64-67 = even-quad dens -> out rows 0-63; 0-3 = odd-quad
    # dens -> out rows 64-127), broadcast to 16 consecutive emb rows each
    e68 = np.zeros((68, 128), f32)
    for i in range(64):
        e68[64 + i // 16, i] = 1.0
        e68[i // 16, 64 + i] = 1.0

    # packed consts [128, 1052]
    cst = np.zeros((128, CSTW), ml_dtypes.bfloat16)
    cst[:, 0:128] = esp2a
    cst[:, 128:256] = esp2b
    cst[:, 256:324] = selE
    cst[:, 324:452] = selO
    cst[0:68, 452:580] = e68
    cst[0, 580:708] = 1.0                        # onesrow
    cst[0, 708 + VW * np.arange(GH) + 16] = 1.0  # vseed (cols 708:1116)

    # V interleaved with a ones column per head: block j = [Wv head j | 0]
    Wvil = np.zeros((C, CAUG), f32)
    for j in range(GH):
        Wvil[:, VW * j:VW * j + 16] = Wv[:, 16 * j:16 * j + 16]

    bf16 = ml_dtypes.bfloat16

    def pack(w):
        # [384, cols] -> [128, 3*cols]: col block s = w[128 s : 128 s + 128]
        cols = w.shape[1]
        return np.ascontiguousarray(
            w.reshape(3, 128, cols).transpose(1, 0, 2).reshape(128, 3 * cols)
            .astype(bf16))

    wko = np.concatenate([pack(Wk).reshape(128, 3, C),
                          pack(Wo).reshape(128, 3, C)], axis=2)
    return dict(
        wko=np.ascontiguousarray(wko.reshape(128, 3 * 2 * C)),
        wvil=pack(Wvil), cst=cst,
        e68=np.ascontiguousarray(e68),
        _wq=pack(Wq).reshape(128, 3, C),
    )


def _rope_cache(pos, rope_freqs):
    # cos/sin patterns [128, 256]: row r (mod 64) = 16 m + d -> head h = m%2,
    # freq index d//2; two stacked 64-row copies
    f32 = np.float32
    theta = np.einsum('ns,shf->nhf', pos.astype(f32), rope_freqs.astype(f32))
    r = np.arange(64)
    h = (r // 16) % 2
    f = (r % 16) // 2
    cpat = np.cos(theta[:, h, f]).T.astype(f32)   # [64, 256]
    spat = np.sin(theta[:, h, f]).T.astype(f32)
    import ml_dtypes
    cs = np.empty((128, 2 * NPC), ml_dtypes.bfloat16)
    cs[0:64, 0:NPC] = cpat
    cs[64:128, 0:NPC] = cpat
    cs[0:64, NPC:] = spat
    cs[64:128, NPC:] = spat
    return cs


def _build_nc():
    import concourse.bacc as bacc
    import concourse.tile as tile
    import concourse.mybir as mybir
    from contextlib import ExitStack

    f32 = mybir.dt.float32
    fmm = mybir.dt.float32r
    fb = mybir.dt.bfloat16
    AF = mybir.ActivationFunctionType

    nc = bacc.Bacc("TRN2", target_bir_lowering=False)

    # xtw: per-slab [xT_s | Wq_s] so one DMA feeds the first projections;
    # wko: per-slab [Wk_s | Wo_s]
    xtw_d = nc.dram_tensor("xtw", [128, 3 * (NPC + C)], fb, kind="ExternalInput")
    cs_d = nc.dram_tensor("cs", [128, 2 * NPC], fb, kind="ExternalInput")
    wko_d = nc.dram_tensor("wko", [128, 3 * 2 * C], fb, kind="ExternalInput")
    wvil_d = nc.dram_tensor("wvil", [128, 3 * CAUG], fb, kind="ExternalInput")
    cst_d = nc.dram_tensor("cst", [128, CSTW], fb, kind="ExternalInput")
    e68_d = nc.dram_tensor("e68", [68, 128], f32, kind="ExternalInput")
    y_d = nc.dram_tensor("y", [NPC, C], f32, kind="ExternalOutput")

    ctx = ExitStack()
    with tile.TileContext(nc) as tc, ctx:
        consts = ctx.enter_context(tc.tile_pool(name="consts", bufs=1))
        sb = ctx.enter_context(tc.tile_pool(name="sbuf", bufs=1))
        # general psum: shared tag -> recycled 1-bank slots
        ps_gp = ctx.enter_context(tc.tile_pool(name="ps_gp", bufs=2, space="PSUM"))
        ps_att = ctx.enter_context(tc.tile_pool(name="ps_att", bufs=1, space="PSUM"))
        ps_av = ctx.enter_context(tc.tile_pool(name="ps_av", bufs=1, space="PSUM"))

        def gpt(shape):
            return ps_gp.tile(shape, f32, tag="pp", name="pp")

        # ---- input DMAs: few large single-descriptor transfers, ordered by
        # first use, split across the two queues ----
        xtw = consts.tile([128, 3, NPC + C], fb, tag="xtw")
        nc.sync.dma_start(out=xtw,
                          in_=xtw_d.rearrange("p (s e) -> p s e", s=3))
        cs = consts.tile([128, 2, NPC], fb, tag="cs")
        nc.sync.dma_start(out=cs, in_=cs_d.rearrange("p (s e) -> p s e", s=2))
        wko = consts.tile([128, 3, 2 * C], fb, tag="wko")
        nc.scalar.dma_start(out=wko,
                            in_=wko_d.rearrange("p (s e) -> p s e", s=3))
        cst = consts.tile([128, CSTW], fb, tag="cst")
        nc.scalar.dma_start(out=cst, in_=cst_d[:])
        wvil = consts.tile([128, 3, CAUG], fb, tag="wvil")
        nc.sync.dma_start(out=wvil,
                          in_=wvil_d.rearrange("p (s e) -> p s e", s=3))
        e68 = consts.tile([68, 128], f32, tag="e68")
        nc.scalar.dma_start(out=e68, in_=e68_d[:])

        cosf = cs[:, 0, :]
        sinf = cs[:, 1, :]
        esp2a = cst[:, 0:128]
        esp2b = cst[:, 128:256]
        selE = cst[:, 256:324]
        selO = cst[:, 324:452]
        onesrow = cst[0:1, 580:708]
        vseed = cst[0:1, 708:708 + CAUG]

        # ---- projections (transposed) + RoPE + spread, per 128-row m-slab.
        # spread = esp2a^T (qt*cos) + esp2b^T (qt*sin), accumulated in psum.
        # q/k interleaved per slab so the PE has projection work to do while
        # the DVE muls feed the spread matmuls; both 64-row halves land in
        # one single-bank [128, 512] psum tile -> one scalar copy each, and
        # the PE never waits on a copy for a psum slot.
        # Pair-0 scores for slab m-1 ride along after slab m.
        def proj(w, woff, m):
            ps = gpt([128, NPC])
            for k in range(3):
                nc.tensor.matmul(
                    out=ps,
                    lhsT=w[:, k, woff + 128 * m:woff + 128 * m + 128],
                    rhs=xtw[:, k, 0:NPC],
                    start=(k == 0), stop=(k == 2))
            return ps

        # qsp/ksp: per slab m one [128, 512] tile; tilei 2m+half -> col half
        def scol(gh):
            return 512 * (gh % 4) + 64 * (gh // 4)

        def sview(lst, tilei):
            return lst[tilei // 2][:, 256 * (tilei % 2):256 * (tilei % 2) + 256]

        def emit_scores(stp, pair, tiles):
            # Concurrent row-tiled matmuls MUST land in distinct PSUM banks:
            # head gh -> bank TILE stp[gh%4], col 64*(gh//4), rows 64*g01.
            # Per-bank tiles (vs one 4-bank tile) let pair-1 scores start as
            # soon as that bank's pair-0 exp has read it.
            for tilei in tiles:
                for slot in range(4):
                    gh = 4 * tilei + slot
                    lo = 32 * slot
                    col = 64 * (gh // 4)
                    for g01 in range(2):
                        g = 2 * pair + g01
                        nc.tensor.matmul(
                            out=stp[slot][64 * g01:64 * g01 + 64,
                                          col:col + 64],
                            lhsT=sview(ksp, tilei)[lo:lo + 16, 64 * g:64 * g + 64],
                            rhs=sview(qsp, tilei)[lo:lo + 16, 64 * g:64 * g + 64],
                            start=True, stop=True,
                            tile_position=(lo, 64 * g01))

        qsp, ksp = [], []
        vau = []
        stp0 = None
        for m in range(3):
            ab = {}
            for tag, woff in (("q", NPC), ("k", 0)):
                ps = proj(xtw if tag == "q" else wko, woff, m)
                a = sb.tile([128, NPC], fb, tag=f"ra{tag}{m}")
                b = sb.tile([128, NPC], fb, tag=f"rb{tag}{m}")
                nc.vector.tensor_mul(out=a, in0=ps, in1=cosf)
                nc.vector.tensor_mul(out=b, in0=ps, in1=sinf)
                ab[tag] = (a, b)
            for tag, lst in (("q", qsp), ("k", ksp)):
                a, b = ab[tag]
                # bf16 (not f32r): score matmuls use tile_position dst
                # offsets that are invalid for f32r operands
                t = sb.tile([128, 2 * NPC], fb, tag=f"sps{tag}{m}")
                for half in range(2):
                    hs = slice(64 * half, 64 * half + 64)
                    csl = slice(NPC * half, NPC * half + NPC)
                    # q spreads borrow the (idle until AV) ps_av banks so no
                    # spread matmul ever waits on a psum->sbuf copy for a slot
                    if tag == "q":
                        sp = ps_av.tile([128, NPC], f32, tag=f"av{half}",
                                        name="spq")
                    else:
                        sp = gpt([128, NPC])
                    nc.tensor.matmul(out=sp, lhsT=esp2a[hs, :],
                                     rhs=a[hs, :], start=True, stop=False)
                    nc.tensor.matmul(out=sp, lhsT=esp2b[hs, :],
                                     rhs=b[hs, :], start=False, stop=True)
                    if m == 2:
                        # last slab's copies on the DVE so the scalar engine
                        # sits parked at the score exps when scores finish
                        nc.vector.tensor_copy(out=t[:, csl], in_=sp)
                    else:
                        nc.scalar.activation(out=t[:, csl], in_=sp,
                                             func=AF.Copy)
                lst.append(t)
            if m == 0:
                # V_aug [256, 408] untransposed (+ ones cols via K=1 matmul);
                # fills PE slack while slab-0's spread copies drain
                for i in range(2):
                    ps = gpt([128, CAUG])
                    for k in range(3):
                        nc.tensor.matmul(
                            out=ps,
                            lhsT=xtw[:, k, 128 * i:128 * i + 128],
                            rhs=wvil[:, k, :],
                            start=(k == 0), stop=False)
                    nc.tensor.matmul(
                        out=ps, lhsT=onesrow, rhs=vseed,
                        start=False, stop=True)
                    t = sb.tile([128, CAUG], fb, tag=f"vau{i}")
                    nc.vector.tensor_copy(out=t, in_=ps)
                    vau.append(t)
            if m > 0:
                # pair-0 scores for slab m-1 ride along behind slab m, so
                # their qsp/ksp copies are long done when the PE gets here
                if stp0 is None:
                    stp0 = [ps_att.tile([128, 512], f32, tag=f"sb{b}", name="stp")
                            for b in range(4)]
                emit_scores(stp0, 0, (2 * (m - 1), 2 * (m - 1) + 1))

        # ---- remaining scores + exp per graph-pair ----
        emit_scores(stp0, 0, (4, 5))
        expst = []
        et0 = sb.tile([128, 4 * 512], fb, tag="expst0")
        for b in range(4):
            nc.scalar.activation(
                out=et0[:, 512 * b:512 * b + 384],
                in_=stp0[b][:, 0:384],
                func=AF.Exp, scale=0.25)
        expst.append(et0)
        stp1 = [ps_att.tile([128, 512], f32, tag=f"sb{b}", name="stp")
                for b in range(4)]
        emit_scores(stp1, 1, range(6))
        et1 = sb.tile([128, 4 * 512], fb, tag="expst1")
        for b in range(4):
            nc.scalar.activation(
                out=et1[:, 512 * b:512 * b + 384],
                in_=stp1[b][:, 0:384],
                func=AF.Exp, scale=0.25)
        expst.append(et1)

        # ---- AV (+den row): per (quad, parity) [128, 128] psum tiles so the
        # two concurrent row groups (graph parities) use distinct banks;
        # head slot 32*(gh%4) rows, col 64*(g//2).
        # Then unspread via one sel matmul: rows 0-3 = den, 4-67 = compact out.
        avsb = sb.tile([128, 6 * 256], fb, tag="avsb")
        og = [sb.tile([128, NPC], f32, tag=f"og{m}", name="og") for m in range(3)]
        # lden: ln(den); quad pair p -> cols 256p; even quad dens at rows
        # 64-67, odd at rows 0-3 (parity-matched to the Ln source partitions).
        # Rows 4-63 are never written but read by the e68 matmul: zero once.
        # (A DVE reciprocal costs ~1.7us flat, so normalization goes through
        # scalar Ln -> e68 broadcast matmul -> scalar Exp(scale=-1) instead.)
        lden = sb.tile([68, 3 * NPC], f32, tag="lden")
        nc.vector.memset(lden, 0.0)
        # persistent per-parity AV psum tiles with 4 column regions (qd % 4)
        # -> 4-deep quad pipelining within 2 banks, no recycle stalls
        avt = [ps_av.tile([128, 512], f32, tag=f"av{g01}", name="av")
               for g01 in range(2)]
        nc.vector.memset(avt[0], 0.0)
        nc.vector.memset(avt[1], 0.0)
        ups_of = {}

        def finish(qd):
            # unspread + Ln + (pair boundary) og copies for quad qd; called
            # one quad late so the PE never idles waiting on the avsb casts
            odd = qd % 2
            cq = slice(256 * qd, 256 * qd + 256)
            ups = gpt([128, NPC])
            nc.tensor.matmul(out=ups[0:68, :] if not odd else ups,
                             lhsT=selO if odd else selE, rhs=avsb[:, cq],
                             start=True, stop=True)
            ups_of[qd] = ups
            dhs = slice(64 - 64 * odd, 68 - 64 * odd)
            rp = slice(NPC * (qd // 2), NPC * (qd // 2) + NPC)
            nc.scalar.activation(out=lden[dhs, rp], in_=ups[dhs, :],
                                 func=AF.Ln)
            if odd:
                # node columns stay in (g01, pair, i) order (host gather
                # undoes it)
                nc.vector.tensor_copy(out=og[qd // 2][0:64, :],
                                      in_=ups_of[qd - 1][0:64, :])
                nc.vector.tensor_copy(out=og[qd // 2][64:128, :],
                                      in_=ups[64:128, :])

        def av_half(qd, pair):
            reg = 128 * (qd % 4)
            for g01 in range(2):
                g = 2 * pair + g01
                lo = 64 * g01
                for a in range(4):
                    gh = 4 * qd + a
                    nc.tensor.matmul(
                        out=avt[g01][32 * a:32 * a + VW,
                                     reg + 64 * pair:reg + 64 * pair + 64],
                        lhsT=vau[pair][lo:lo + 64, VW * gh:VW * gh + VW],
                        rhs=expst[pair][lo:lo + 64, scol(gh):scol(gh) + 64],
                        start=True, stop=True,
                        tile_position=(lo, 32 * a))

        # pair-0 halves of quads 0-3 first: the PE grinds these while the
        # scalar engine is still serially producing the pair-1 exps
        for qd in range(4):
            av_half(qd, 0)
        for qd in range(6):
            if qd >= 4:
                av_half(qd, 0)
            av_half(qd, 1)
            reg = 128 * (qd % 4)
            for g01 in range(2):
                nc.vector.tensor_copy(
                    out=avsb[:, 256 * qd + 128 * g01:256 * qd + 128 * g01 + 128],
                    in_=avt[g01][:, reg:reg + 128])
            if qd > 0:
                finish(qd - 1)
        finish(5)

        # ---- normalize + y = O_norm @ Wo ----
        onrm = []
        for m in range(3):
            rt = gpt([128, NPC])
            rp = slice(NPC * m, NPC * m + NPC)
            nc.tensor.matmul(out=rt, lhsT=e68, rhs=lden[:, rp],
                             start=True, stop=True)
            ert = sb.tile([128, NPC], f32, tag=f"ert{m}")
            nc.scalar.activation(out=ert, in_=rt, func=AF.Exp, scale=-1.0)
            t = sb.tile([128, NPC], fb, tag=f"onrm{m}")
            nc.vector.tensor_mul(out=t, in0=og[m], in1=ert)
            onrm.append(t)

        for i in range(2):
            yps = gpt([128, C])
            for m in range(3):
                nc.tensor.matmul(
                    out=yps,
                    lhsT=onrm[m][:, 128 * i:128 * i + 128],
                    rhs=wko[:, m, C:2 * C],
                    start=(m == 0), stop=(m == 2))
            ysb = sb.tile([128, C], f32, tag=f"ysb{i}", name="ysb")
            if i == 0:
                nc.vector.tensor_copy(out=ysb, in_=yps)
            else:
                # scalar's Copy-table load hides behind the i=0 y matmuls
                nc.scalar.activation(out=ysb, in_=yps, func=AF.Copy)
            # contiguous DRAM write; the host gather undoes the node order
            # (y-tile i covers g01 == i, rows (pair, i64))
            nc.sync.dma_start(
                out=y_d.rearrange("(i r) e -> i r e", i=2)[i],
                in_=ysb)

    nc.compile()
    return nc


def _get_nc():
    if "nc" not in _CACHE:
        _CACHE["nc"] = _build_nc()
    return _CACHE["nc"]


def make_in_maps(inputs):
    x = np.asarray(inputs["x"], np.float32)
    pos = np.asarray(inputs["pos"], np.float32)
    freqs = np.asarray(inputs["rope_freqs"], np.float32)
    prep = _host_prep(np.asarray(inputs["Wq"], np.float32),
                      np.asarray(inputs["Wk"], np.float32),
                      np.asarray(inputs["Wv"], np.float32),
                      np.asarray(inputs["Wo"], np.float32),
                      freqs)
    in_maps = []
    import ml_dtypes
    wqp = prep.pop("_wq")
    for c in range(NCORES):
        sl = slice(c * NPC, (c + 1) * NPC)
        m = dict(prep)
        xs = x[sl].T                                # [384, 256]
        xTp = (xs.reshape(3, 128, NPC).transpose(1, 0, 2)
               .astype(ml_dtypes.bfloat16))         # [128, 3, 256]
        m["xtw"] = np.ascontiguousarray(
            np.concatenate([xTp, wqp], axis=2).reshape(128, 3 * (NPC + C)))
        m["cs"] = _rope_cache(pos[sl], freqs)
        in_maps.append(m)
    return in_maps


def gather(res):
    """Assemble the full [N, C] output; undoes the per-core (g01, pair, i64)
    node-column order the kernel keeps for contiguous DRAM writes."""
    outs = []
    for c in range(NCORES):
        yr = np.asarray(res.results[c]["y"], np.float32)   # [256, 384] raw
        outs.append(yr.reshape(2, 2, 64, C).transpose(1, 0, 2, 3).reshape(NPC, C))
    return np.concatenate(outs, axis=0)


def kernel(**inputs):
    from concourse.bass_utils import run_bass_kernel_spmd

    in_maps = make_in_maps(inputs)

    nc = _get_nc()
    res = run_bass_kernel_spmd(nc, in_maps, core_ids=list(range(NCORES)))
    return gather(res)
